# revision 73
# baseline (speedup 1.0000x reference)
"""Multi-head causal self-attention on 8 trn2 NeuronCores.

Problem: x[4, 2048, 1024], 16 heads of 64 dims, causal softmax attention,
torch-Linear style projections (y = x @ W.T + b).

Sharding: core c = (batch b = c // 2, head-group g = c % 2). Each core
computes the attention output for batch b over heads [8g, 8g+8) and the
partial output projection for those heads' 512 value dims. The host sums
the two head-group partials per batch (the "all-reduce after W_O" of
tensor parallelism, done during unshard) and adds the rank-1 bias
corrections (bv @ Wo.T + bo), which commute with attention because
softmax rows sum to 1.

Precision plan (cost model charges matmuls by output free-size x
cycles/row only; fp8e4+DoubleRow = 0.5 cyc/row with 2x128 contraction
per instruction, fp16 = 1.0, fp32 = 4.0):
  - Q/K projections run fp8e4 DoubleRow (4x over fp32r): host ships
    x8 = fp8(4x) and wq8/wk8 = fp8(8 W.T); the 32x scale keeps fp8
    operands out of e4m3's subnormal range and folds into the exp scale
    (0.125/1024). Projection psums drain to fp16 Q~/K~ (scaled by 32).
  - The V projection runs three fp8 DoubleRow chains with host-built
    residuals — (x8+xr8)@(wv8+wvr8) minus the negligible cross term —
    i.e. fp16-grade accuracy at 0.75x the fp16 matmul cost. V rides
    32x-scaled through P@V and the tail; Wo is host-divided by 32.
  - Scores, P@V, and W_O stay fp16: fp8 there pushes the end-to-end
    max-rel error to ~1.7e-2 against the 2e-2 gate (measured in a numpy
    model of the full pipeline); this config measures 1.245e-2 on
    hardware, dominated by the fp8 Q/K projection error, which enters
    only through the softmax exponent (x0.125 discount).
  - attn/attnT/y all fp16: fp16 PE transposes cost 1 cyc/row (vs 2 for
    fp32) and write an fp16 bitcast view of the fp32 psum fill tile;
    y DMAs out as fp16 and the host accumulates partials in fp32.

Device layouts (per core):
  x8T  [1024, 2048] fp8  4 x[b].T      xr8T  [1024, 2048] fp8 residual
  wq8T [1024, 512]  fp8  8 Wq[g].T     (wk8T same)
  wv8T/wvr8T [1024, 512] fp8           wo16T [512, 1024] fp16 Wo.T[g]/32
  bq   [512] f32 32 bq[g] shard        y [2048, 1024] fp16 partial out

On-chip pipeline, interleaved over 512-wide column chunks:
  - Q~/K~ [dq, T] fp16 via fp8 DoubleRow weight-stationary matmuls
    (Q kept per-window only); V [T-slice, dv] fp16 via the residual
    DoubleRow chains, stored per head with a ones column so the P@V'
    matmul also produces the softmax denominators.
  - Scores transposed per head, s_T[k, q] = K~ Q~.T (fp16, scaled 1024x).
    Score pairs land in one 2-bank PSUM tile so a single ACT instruction
    exponentiates two k-chunks (ACT per-instruction overhead matters:
    ~185ns on ~850ns of work), emitting fp16 P.
  - The causal mask is a multiplicative 0/1 square applied after exp
    (off the scores->exp critical chain).
  - P@V' in fp16 with the exp tile stationary, sub-q-outer (one PSUM
    bank per accumulation group), lagging one head behind scores/exp so
    it never waits on ACT; projection/tail work fills the PE between
    heads. 1/denominator folds into the PSUM drain (vector engine).
  - The attention output is PE-transposed (fp16) per sub-q chunk and fed
    to the fp16 W_O matmul.

Engine budget per core (cost model): PE ~157us busy (Q/K 13.7 + V 20.5,
scores 61, P@V 29, transposes 3.4, W_O 27), ACT ~157us (exp 152 + final
tail drains), DVE ~82us (psum drains, rescales), Pool ~46us (causal
masks — SBUF-only, so gpsimd can host them and they stay out of DVE's
in-order queue where they'd delay drains), DMA ~35us (7.5MB in + 4MB
out). PE and ACT are balanced co-bottlenecks at ~82% duty; modeled span
190.1us = busy + startup ramp (~9), early-window ACT starvation while
projections/V fill the PE (~9), and the endgame drain chain (~9).
Cross-window score lookahead (the PULL map) recovers most of the early
starvation in the cost model but was nondeterministically wrong on
hardware (NaNs on some runs) and is disabled.
Measured end-to-end rel err 1.245e-2 (gate 2e-2), deterministic across
runs; numpy precision model agrees to 3 digits (1.248e-2).
"""

from contextlib import ExitStack

import numpy as np

import concourse.bass as bass
import concourse.mybir as mybir
import concourse.tile as tile
from concourse import bacc
from concourse.masks import make_identity

F32 = mybir.dt.float32
F16 = mybir.dt.float16
F8 = mybir.dt.float8e4
DR = mybir.MatmulPerfMode.DoubleRow
Exp = mybir.ActivationFunctionType.Exp
Identity = mybir.ActivationFunctionType.Identity

SX = 4.0               # host scale on x8 (keeps fp8 quant relative)
SW = 8.0               # host scale on wq8/wk8 (lifts weights out of subnormals)
QK_SCALE = SX * SW     # Q~/K~ are scaled by this
EXP_SCALE = 0.125 / (QK_SCALE * QK_SCALE)

D = 1024          # model dim
T = 2048          # sequence length
BATCH = 4
NH = 16           # total heads
DH = 64           # head dim
HLOC = 8          # heads per core
DSH = 512         # value dims per core (HLOC * DH)
N_CORES = 8

TC = T // 512     # 4 column tiles of 512
KC = T // 128     # 16 k chunks of 128
DC = D // 128     # 8 contraction chunks for the QKV projections


def _build(ablate=()):
    """ablate: subset of {"pv", "exp", "scores", "mask", "rescale", "tail"}
    — drop those instruction groups (timing studies only; output garbage)."""
    nc = bacc.Bacc("TRN2", target_bir_lowering=False, debug=False,
                   num_devices=N_CORES)
    x8T = nc.dram_tensor("x8T", [D, T], F8, kind="ExternalInput").ap()
    xr8T = nc.dram_tensor("xr8T", [D, T], F8, kind="ExternalInput").ap()
    wq8T = nc.dram_tensor("wq8T", [D, DSH], F8, kind="ExternalInput").ap()
    wk8T = nc.dram_tensor("wk8T", [D, DSH], F8, kind="ExternalInput").ap()
    wv8T = nc.dram_tensor("wv8T", [D, DSH], F8, kind="ExternalInput").ap()
    wvr8T = nc.dram_tensor("wvr8T", [D, DSH], F8, kind="ExternalInput").ap()
    wo16T = nc.dram_tensor("wo16T", [DSH, D], F16, kind="ExternalInput").ap()
    bq = nc.dram_tensor("bq", [DSH], F32, kind="ExternalInput").ap()
    bk = nc.dram_tensor("bk", [DSH], F32, kind="ExternalInput").ap()
    y = nc.dram_tensor("y", [T, D], F16, kind="ExternalOutput").ap()

    with tile.TileContext(nc) as tc, ExitStack() as ctx:
        singles = ctx.enter_context(tc.tile_pool(name="singles", bufs=1))
        wpool = ctx.enter_context(tc.tile_pool(name="wpool", bufs=1))
        xtpool = ctx.enter_context(tc.tile_pool(name="xtpool", bufs=2))
        qtpool = ctx.enter_context(tc.tile_pool(name="qt", bufs=2))
        attnp = ctx.enter_context(tc.tile_pool(name="attnp", bufs=4))
        attnTp = ctx.enter_context(tc.tile_pool(name="attnTp", bufs=3))
        # exp tiles live from scores until their (queued) P@V consumes them;
        # the lookahead pend-queue peaks around 24 pair-tiles in window 2
        exp_pool = ctx.enter_context(tc.tile_pool(name="exp", bufs=32))
        small = ctx.enter_context(tc.tile_pool(name="small", bufs=8))
        ybuf = ctx.enter_context(tc.tile_pool(name="ybuf", bufs=4))
        # PSUM: 4 (two double-bank score tiles: exp reads a k-chunk PAIR in
        # one scalar-engine instruction) + 2 (PV accumulators, sub-q-outer)
        # + 2 (fill: projection groups, attn transposes, W_O groups — all
        # emission-interleaved filler work)
        ps_s = ctx.enter_context(tc.tile_pool(name="ps_s", bufs=2, space="PSUM"))
        ps_pv = ctx.enter_context(tc.tile_pool(name="ps_pv", bufs=2, space="PSUM"))
        ps_fill = ctx.enter_context(tc.tile_pool(name="ps_fill", bufs=2, space="PSUM"))

        KT_t = singles.tile([128, 4, T], F16)       # [dk%128, dk//128, t], x32
        Vp_t = singles.tile([128, KC, HLOC, DH + 1], F16)  # [t%128, t//128, h, dv+1]
        ident_t = singles.tile([128, 128], F16)
        mask_t = singles.tile([128, 128], F16)      # 0/1 causal square
        bq_t = singles.tile([128, 4], F32)
        bk_t = singles.tile([128, 4], F32)

        make_identity(nc, ident_t)
        nc.vector.memset(Vp_t[:, :, :, DH:DH + 1], 1.0)
        nc.gpsimd.memset(mask_t, 1.0)
        # s_T layout [k, q]: multiplicative 0/1 causal mask for the 128x128
        # diagonal square, applied to exp(s) AFTER the exp so the mask sits
        # off the scores->exp chain (exp(s)*0 == exp(s-1e6)). Keep 1.0
        # where (qq - kk) >= 0, else 0. (is_le is unimplemented in walrus
        # codegen, hence the negated is_ge form.)
        nc.gpsimd.affine_select(
            out=mask_t, in_=mask_t,
            compare_op=mybir.AluOpType.is_ge,
            fill=0.0,
            base=0,
            pattern=[[1, 128]],
            channel_multiplier=-1,
        )

        # Wq/Wk live c-major ([dq-chunk, contraction-chunk, col]) so the
        # first Q/K projection groups (c=0) complete after a quarter of the
        # weight bytes land — the first scores and exp start earlier
        wq_t = wpool.tile([128, 4, DC, 128], F8)
        wk_t = wpool.tile([128, 4, DC, 128], F8)
        wv_t = wpool.tile([128, DC, DSH], F8)
        wvr_t = wpool.tile([128, DC, DSH], F8)
        wo_t = wpool.tile([128, 4, D], F16)
        wq8T_r = wq8T.rearrange("(d p) (c j) -> p c d j", p=128, c=4)
        wk8T_r = wk8T.rearrange("(d p) (c j) -> p c d j", p=128, c=4)
        wv8T_r = wv8T.rearrange("(d p) j -> p d j", p=128)
        wvr8T_r = wvr8T.rearrange("(d p) j -> p d j", p=128)
        wo16T_r = wo16T.rearrange("(c p) j -> p c j", p=128)
        x8T_r = x8T.rearrange("(d p) t -> p d t", p=128)
        xr8T_r = xr8T.rearrange("(d p) t -> p d t", p=128)

        # emission order sets DMA/engine priority (each dma_start holds the
        # HWDGE descriptor generator ~625ns, so transfers are coalesced to
        # window/tensor granularity): x8(0) + first Wq/Wk block feed the
        # first scores; the V-path inputs are first needed at PV(h0)
        xt8_0 = xtpool.tile([128, DC, 512], F8, tag="xt8", name="xt8")
        nc.sync.dma_start(out=wq_t[:, 0], in_=wq8T_r[:, 0])
        nc.sync.dma_start(out=wk_t[:, 0], in_=wk8T_r[:, 0])
        # one DMA for the whole first x window: HWDGE descriptor generation
        # (625ns per dma_start, serialized) dominates the transfer time, so
        # fewer DMAs reach the first Q/K matmul sooner than split ones
        nc.sync.dma_start(out=xt8_0, in_=x8T_r[:, :, 0:512])
        nc.sync.dma_start(out=bq_t, in_=bq.rearrange("(c p) -> p c", p=128))
        nc.sync.dma_start(out=bk_t, in_=bk.rearrange("(c p) -> p c", p=128))
        for c in range(1, 4):
            nc.sync.dma_start(out=wq_t[:, c], in_=wq8T_r[:, c])
            nc.sync.dma_start(out=wk_t[:, c], in_=wk8T_r[:, c])
        xtr8_0 = xtpool.tile([128, DC, 512], F8, tag="xtr8", name="xtr8")
        nc.sync.dma_start(out=xtr8_0, in_=xr8T_r[:, :, 0:512])
        nc.sync.dma_start(out=wv_t, in_=wv8T_r)
        nc.sync.dma_start(out=wvr_t, in_=wvr8T_r)

        def proj_steps(w, box):
            """Closures emitting projection work for chunk w, finest-grain
            first: xt/qt alloc, Q groups (the attention window needs them
            first), K groups, then V groups — matching DMA data arrival so
            the PE's static instruction order never head-of-line blocks on
            a later weight load. box["qt"] is set by the first step."""
            steps = []

            def alloc(w=w):
                if w == 0:
                    xt8, xtr8 = xt8_0, xtr8_0
                else:
                    xt8 = xtpool.tile([128, DC, 512], F8, tag="xt8", name="xt8")
                    nc.sync.dma_start(out=xt8,
                                      in_=x8T_r[:, :, 512 * w:512 * (w + 1)])
                    xtr8 = xtpool.tile([128, DC, 512], F8, tag="xtr8", name="xtr8")
                    nc.sync.dma_start(out=xtr8,
                                      in_=xr8T_r[:, :, 512 * w:512 * (w + 1)])
                box["xt8"], box["xtr8"] = xt8, xtr8
                box["qt"] = qtpool.tile([128, 4, 512], F16, tag="qt", name="qt_w")
            steps.append(alloc)

            def qkstep(c, w_t, dst, bias_t, act_drain=False, w=w):
                xt8 = box["xt8"]
                psp = ps_fill.tile([128, 512], F32, tag="fill", name="psqk")
                for dp in range(DC // 2):
                    nc.tensor.matmul(
                        psp,
                        lhsT=w_t[:, c, 2 * dp:2 * dp + 2, :],
                        rhs=xt8[:, 2 * dp:2 * dp + 2, :],
                        start=(dp == 0), stop=(dp == DC // 2 - 1),
                        perf_mode=DR,
                    )
                # (an ACT Identity drain for startup parallelism was tried
                # and lost: bass lazily emits a 1.3us LoadActFuncSet before
                # the first non-Exp activation, right on the startup path)
                nc.vector.tensor_scalar_add(dst, psp, bias_t[:, c:c + 1])

            def vstep(s, w=w):
                # V in three fp8 DoubleRow chains with host-built residuals:
                # (x8+xr8)@(wv8+wvr8) minus the negligible xr8@wvr8 cross
                # term — 12 DR matmuls (3072 cyc) vs fp16's 8x512 (4096).
                # The result is 32x-scaled; Wo is host-scaled by 1/32.
                xt8, xtr8 = box["xt8"], box["xtr8"]
                psv = ps_fill.tile([128, 512], F32, tag="fill", name="psv")
                chains = ((xt8, wv_t), (xtr8, wv_t), (xt8, wvr_t))
                for dp in range(DC // 2):
                    for ci, (xa, wa) in enumerate(chains):
                        nc.tensor.matmul(
                            psv,
                            lhsT=xa[:, 2 * dp:2 * dp + 2, 128 * s:128 * (s + 1)],
                            rhs=wa[:, 2 * dp:2 * dp + 2, :],
                            start=(dp == 0 and ci == 0),
                            stop=(dp == DC // 2 - 1 and ci == 2),
                            perf_mode=DR,
                        )
                nc.vector.tensor_copy(
                    Vp_t[:, 4 * w + s, :, 0:DH],
                    psv.rearrange("p (h v) -> p h v", h=HLOC),
                )

            for c in range(4):
                steps.append(lambda c=c: qkstep(
                    c, wq_t, box["qt"][:, c, :], bq_t))
                steps.append(lambda c=c, w=w: qkstep(
                    c, wk_t, KT_t[:, c, 512 * w:512 * (w + 1)], bk_t))
            vsteps = [lambda s=s: vstep(s) for s in range(4)]
            return steps, vsteps

        def emit_scores_exp(w, h, qt_w):
            kmax = 4 * (w + 1)
            ch, po = h // 2, (h % 2) * 64
            # scores for a PAIR of k-chunks land in one 2-bank PSUM tile so
            # a single scalar-engine instruction exponentiates both (ACT
            # per-instruction overhead is the attention loop's scarcest
            # resource). All of the window's exp tiles stay live so the PV
            # loop can run sub-q-outer, one head behind.
            ex_buf = []
            for jp in range(kmax // 2):
                pssb = ps_s.tile([128, 2, 512], F32, tag="pss", name="pss")
                exb = exp_pool.tile([128, 2, 512], F16, tag="ex", name="ex")
                rel0 = 2 * jp - 4 * w
                # both matmuls write from the PAIR's first live column (the
                # second diag chunk's extra 128 columns are garbage that exp
                # covers but PV never reads — writing them keeps the paired
                # exp's input region fully initialized)
                q0 = max(rel0, 0) * 128
                for sub in range(2):
                    j = 2 * jp + sub
                    if "scores" not in ablate:
                        nc.tensor.matmul(
                            pssb[:, sub, q0:],
                            lhsT=KT_t[po:po + 64, ch, 128 * j:128 * (j + 1)],
                            rhs=qt_w[po:po + 64, ch, q0:],
                            start=True, stop=True,
                        )
                # pairs are both-full or both-diagonal (diag chunks are the
                # last 4 and 4w is even). For a diag pair the exp covers
                # [128*rel0:512] of both chunks; chunk rel0+1's columns
                # [128*rel0:128*(rel0+1)] are garbage, but PV of sub-q i
                # only reads chunks with rel <= i, so they're never used.
                e0 = max(rel0, 0) * 128
                if "exp" not in ablate:
                    nc.scalar.activation(out=exb[:, :, e0:],
                                         in_=pssb[:, :, e0:],
                                         func=Exp, scale=EXP_SCALE)
                if "mask" not in ablate:
                    for sub in range(2):
                        rel = 2 * jp + sub - 4 * w
                        if rel >= 0:
                            q0 = rel * 128
                            # zero exp(s) above the diagonal; only PV of
                            # sub-q i == rel reads this square. On gpsimd:
                            # Pool is idle and this keeps the mask out of
                            # DVE's in-order queue, where it would delay
                            # the Q/K/V psum drains behind it
                            nc.gpsimd.tensor_mul(
                                exb[:, sub, q0:q0 + 128],
                                exb[:, sub, q0:q0 + 128], mask_t)
                ex_buf.append((exb, 0))
                ex_buf.append((exb, 1))
            return ex_buf

        def emit_pv(w, h, ex_buf, attn_t, after_i=None):
            # P@V', one 128-query sub-chunk at a time: each accumulation
            # group owns one PSUM bank (bank-granular zero regions) and
            # only 2 are in flight. after_i: per-sub-q closures (the final
            # head's tail pieces) emitted right after each rescale so the
            # endgame pipeline starts before the whole head is done.
            for i in range(4):
                pso = ps_pv.tile([128, DH + 1], F32, tag="pso", name="pso")
                if "pv" not in ablate:
                    jlast = 4 * w + i
                    for j in range(jlast + 1):
                        exb, sub = ex_buf[j]
                        nc.tensor.matmul(
                            pso,
                            lhsT=exb[:, sub, 128 * i:128 * (i + 1)],
                            rhs=Vp_t[:, j, h, :],
                            start=(j == 0), stop=(j == jlast),
                        )
                if "rescale" not in ablate:
                    rec = small.tile([128, 1], F32, tag="rec", name="rec")
                    nc.vector.reciprocal(rec, pso[:, DH:DH + 1])
                    # attn = pv_psum * (1/denom), broadcast along dv
                    nc.vector.tensor_mul(
                        attn_t[:, i, DH * h:DH * (h + 1)],
                        pso[:, 0:DH],
                        rec.broadcast_to([128, DH]),
                    )
                if after_i is not None:
                    after_i[i]()

        def emit_tail_pieces(w, attn_t, use_act=False):
            """Transpose + W_O + store for window w as four per-128-query
            closures, consumed one per head so the tail never lumps between
            two heads' scores. Transposes are fp16 (1 cyc/row) into an fp16
            bitcast view of the fp32 fill tile. The final window's drains go
            to the scalar engine (idle once the last exp retires) and its y
            DMAs split per 512-chunk so the last transfer is short."""
            if "tail" in ablate:
                return []
            # final window: atT drains on ACT (idle once exp retires), ysb
            # drains stay on DVE so consecutive pipeline stages alternate
            # engines; transposes borrow the now-idle score psum pool so
            # pieces overlap instead of ping-ponging on ps_fill bufs
            drain = nc.scalar.copy if use_act else nc.vector.tensor_copy

            def piece(i, w=w, attn_t=attn_t):
                atT = attnTp.tile([128, 4, 128], F16, tag="attnT", name="attnT")
                if use_act:
                    pst = ps_s.tile([128, 2, 512], F32, tag="pss",
                                    name="pstE")[:, 0, :]
                else:
                    pst = ps_fill.tile([128, 512], F32, tag="fill", name="pst")
                pst16 = pst.bitcast(F16)  # [128, 1024] fp16 view
                for c in range(4):
                    nc.tensor.transpose(
                        pst16[:, 128 * c:128 * (c + 1)],
                        attn_t[:, i, 128 * c:128 * (c + 1)], ident_t)
                drain(atT, pst16[:, 0:512].rearrange("p (c q) -> p c q", c=4))
                ysb = ybuf.tile([128, 2, 512], F16, tag="ysb", name="ysb")
                for jc in range(2):
                    py = ps_fill.tile([128, 512], F32, tag="fill", name="py")
                    for c in range(4):
                        nc.tensor.matmul(
                            py,
                            lhsT=atT[:, c, :],
                            rhs=wo_t[:, c, 512 * jc:512 * (jc + 1)],
                            start=(c == 0), stop=(c == 3),
                        )
                    # final window: alternate ysb drains DVE/ACT so the two
                    # output chunks of a piece drain in parallel
                    if use_act and jc == 1:
                        nc.scalar.copy(ysb[:, jc, :], py)
                    else:
                        nc.vector.tensor_copy(ysb[:, jc, :], py)
                    if use_act:
                        nc.sync.dma_start(
                            out=y[512 * w + 128 * i:512 * w + 128 * (i + 1),
                                  512 * jc:512 * (jc + 1)],
                            in_=ysb[:, jc, :],
                        )
                if not use_act:
                    nc.sync.dma_start(
                        out=y[512 * w + 128 * i:512 * w + 128 * (i + 1), :],
                        in_=ysb.rearrange("p j q -> p (j q)"),
                    )
            return [lambda i=i: piece(i) for i in range(4)]

        # Driver: a softly-pipelined schedule over 32 (window, head) tasks.
        # P@V trails scores/exp through a pend QUEUE; windows 0/1 pull the
        # next window's first heads' scores+exp into their last slots so the
        # scalar engine's in-order exp stream never starves across window
        # boundaries (early windows are PE-bound, late ones ACT-bound). The
        # queue debt is repaid with double flushes in the late-w2/w3 slots,
        # where exp is long and the PE idles. PE filler placement: Q/K
        # projections for w+1 spread evenly over w's slots; V(w) groups run
        # at w's first two slots (before PV(w,h0)'s flush — PE executes in
        # emission order, so V(w) must precede it); tail pieces defer into
        # windows >= 2.
        PULL = {}              # lookahead disabled: pulls raced on hardware
        box0 = {}
        qk0, v0 = proj_steps(0, box0)
        for s in qk0:          # alloc, Q x4, K x4
            s()
        qt_map = {0: box0["qt"]}
        attn_map = {}
        boxes = {}
        pendq = []             # (w, h, ex_buf, attn_t) awaiting PV, FIFO
        v_now = v0             # V groups for the current window
        v_next = []
        qk_carry = []
        tailq = []             # pending per-sub-q tail closures

        def get_attn(wi):
            if wi not in attn_map:
                attn_map[wi] = attnp.tile([128, 4, DSH], F16, tag="attn",
                                          name="attn_t")
            return attn_map[wi]

        def flush_one():
            pw, ph, pex, pat = pendq.pop(0)
            emit_pv(pw, ph, pex, pat)
            if ph == HLOC - 1:           # window pw fully rescaled
                tailq.extend(emit_tail_pieces(pw, pat))

        for w in range(TC):
            start_h = PULL.get(w - 1, 0)
            own = list(range(start_h, HLOC))
            pulls = list(range(PULL.get(w, 0))) if w + 1 < TC else []
            nslots = len(own)
            qk_steps = list(qk_carry)
            qk_carry = []
            if w == 0:
                qk_steps.append(lambda: nc.sync.dma_start(
                    out=wo_t, in_=wo16T_r))
            if w + 1 < TC:
                box = {}
                nqk, v_next = proj_steps(w + 1, box)
                qk_steps += nqk
                boxes[w + 1] = box
            it = iter(qk_steps)
            n_qk = len(qk_steps)
            done = 0
            for idx, h in enumerate(own):
                last_task = (w == TC - 1 and h == HLOC - 1)
                ex = emit_scores_exp(w, h, qt_map[w])
                # V(w) spreads 2,1,1 over slots 0-2: skipping the slot-1 PV
                # flush (below) pushes PV(w,0) to slot 2, so slot 1 sheds a
                # V group from the PE backlog that delays the exp stream in
                # the PE-bound early windows
                for s in v_now[:1]:      # V(w): one group per slot, 0-3
                    s()
                v_now = v_now[1:]
                target = n_qk * (idx + 1) / nslots
                while done < target:
                    s = next(it, None)
                    if s is None:
                        break
                    s()
                    done += 1
                if last_task:
                    nf = len(pendq)      # drain everything before the tail
                elif idx in (1, 2):
                    # defer PV(w,0) to slot 3: V(w)#4 (emitted at slot 3,
                    # before the flush) must precede it in PE program order
                    # — real silicon only reorders Ldweights, not matmuls
                    nf = 0
                elif idx >= HLOC - 2:
                    nf = 2               # repay the deferred flushes
                else:
                    nf = 1
                for _ in range(nf):
                    if pendq:
                        flush_one()
                if w >= 2 and tailq:
                    tailq.pop(0)()
                if last_task:
                    # final head: pipeline its PV with the window's tail
                    emit_pv(w, h, ex, get_attn(w),
                            after_i=emit_tail_pieces(w, get_attn(w),
                                                     use_act=True) or None)
                else:
                    pendq.append((w, h, ex, get_attn(w)))
                pi = idx - (nslots - len(pulls))
                if 0 <= pi < len(pulls):
                    qt_map[w + 1] = boxes[w + 1]["qt"]
                    ex2 = emit_scores_exp(w + 1, pulls[pi], qt_map[w + 1])
                    pendq.append((w + 1, pulls[pi], ex2, get_attn(w + 1)))
            v_now = v_next
            v_next = []
            if w + 1 < TC:
                qt_map[w + 1] = boxes[w + 1]["qt"]
        for _ in range(len(pendq)):
            flush_one()
        for t in tailq:
            t()
    nc.compile()
    return nc


def shard_inputs(x, Wq, bq, Wk, bk, Wv, bv, Wo, bo):
    """Returns the 8 per-core input maps (host-side dtype/layout prep)."""
    import ml_dtypes

    F8NP = ml_dtypes.float8_e4m3
    in_maps = []
    for c in range(N_CORES):
        b, g = c // 2, c % 2
        sl = slice(DSH * g, DSH * (g + 1))
        xT = np.ascontiguousarray(x[b].T) * SX
        x8 = xT.astype(F8NP)
        wvs = np.ascontiguousarray(Wv[sl, :].T) * SW
        wv8 = wvs.astype(F8NP)
        in_maps.append({
            "x8T": x8,
            "xr8T": (xT - x8.astype(np.float32)).astype(F8NP),
            "wq8T": np.ascontiguousarray(Wq[sl, :].T * SW).astype(F8NP),
            "wk8T": np.ascontiguousarray(Wk[sl, :].T * SW).astype(F8NP),
            "wv8T": wv8,
            "wvr8T": (wvs - wv8.astype(np.float32)).astype(F8NP),
            # Wo pre-divided by the V-path scale (SX*SW) so attn rides
            # 32x-scaled through PV and the tail
            "wo16T": np.ascontiguousarray(Wo.T[sl, :] / (SX * SW)).astype(
                np.float16),
            "bq": np.ascontiguousarray(bq[sl] * QK_SCALE).astype(np.float32),
            "bk": np.ascontiguousarray(bk[sl] * QK_SCALE).astype(np.float32),
        })
    return in_maps


def combine_outputs(results, bv, Wo, bo):
    """Sum head-group partials per batch + rank-1 bias corrections."""
    corr = (bv @ Wo.T + bo).astype(np.float32)  # [D]; exact because softmax
    y = np.empty((BATCH, T, D), dtype=np.float32)  # rows sum to 1
    for b in range(BATCH):
        y[b] = (results[2 * b]["y"].astype(np.float32)
                + results[2 * b + 1]["y"].astype(np.float32) + corr)
    return y


def run_sharded(inputs, trace=False):
    """Build, compile, run on cores 0-7. Returns (y_full, BassKernelResults)."""
    from concourse import bass_utils

    inputs = {k: np.asarray(v, dtype=np.float32) for k, v in inputs.items()}
    nc = _build()
    in_maps = shard_inputs(
        inputs["x"], inputs["Wq"], inputs["bq"], inputs["Wk"], inputs["bk"],
        inputs["Wv"], inputs["bv"], inputs["Wo"], inputs["bo"])
    res = bass_utils.run_bass_kernel_spmd(
        nc, in_maps, list(range(N_CORES)), trace=trace)
    y = combine_outputs(res.results, inputs["bv"], inputs["Wo"], inputs["bo"])
    return y, res


def kernel(**inputs):
    y, _ = run_sharded(inputs, trace=False)
    return y


if __name__ == "__main__":
    rng = np.random.default_rng(0)
    demo = {
        "x": rng.standard_normal((BATCH, T, D), dtype=np.float32),
        "Wq": rng.standard_normal((D, D), dtype=np.float32) * 0.02,
        "bq": np.zeros(D, np.float32),
        "Wk": rng.standard_normal((D, D), dtype=np.float32) * 0.02,
        "bk": np.zeros(D, np.float32),
        "Wv": rng.standard_normal((D, D), dtype=np.float32) * 0.02,
        "bv": np.zeros(D, np.float32),
        "Wo": rng.standard_normal((D, D), dtype=np.float32) * 0.02,
        "bo": np.zeros(D, np.float32),
    }
    out = kernel(**demo)
    print(out.shape, out.dtype)


# revision 76
# speedup vs baseline: 1.0014x; 1.0014x over previous
"""Multi-head causal self-attention on 8 trn2 NeuronCores.

Problem: x[4, 2048, 1024], 16 heads of 64 dims, causal softmax attention,
torch-Linear style projections (y = x @ W.T + b).

Sharding: core c = (batch b = c // 2, head-group g = c % 2). Each core
computes the attention output for batch b over heads [8g, 8g+8) and the
partial output projection for those heads' 512 value dims. The host sums
the two head-group partials per batch (the "all-reduce after W_O" of
tensor parallelism, done during unshard) and adds the rank-1 bias
corrections (bv @ Wo.T + bo), which commute with attention because
softmax rows sum to 1.

Precision plan (cost model charges matmuls by output free-size x
cycles/row only; fp8e4+DoubleRow = 0.5 cyc/row with 2x128 contraction
per instruction, fp16 = 1.0, fp32 = 4.0):
  - Q/K projections run fp8e4 DoubleRow (4x over fp32r): host ships
    x8 = fp8(4x) and wq8/wk8 = fp8(8 W.T); the 32x scale keeps fp8
    operands out of e4m3's subnormal range and folds into the exp scale
    (0.125/1024). Projection psums drain to fp16 Q~/K~ (scaled by 32).
  - The V projection runs three fp8 DoubleRow chains with host-built
    residuals — (x8+xr8)@(wv8+wvr8) minus the negligible cross term —
    i.e. fp16-grade accuracy at 0.75x the fp16 matmul cost. V rides
    32x-scaled through P@V and the tail; Wo is host-divided by 32.
  - Scores, P@V, and W_O stay fp16: fp8 there pushes the end-to-end
    max-rel error to ~1.7e-2 against the 2e-2 gate (measured in a numpy
    model of the full pipeline); this config measures 1.245e-2 on
    hardware, dominated by the fp8 Q/K projection error, which enters
    only through the softmax exponent (x0.125 discount).
  - attn/attnT/y all fp16: fp16 PE transposes cost 1 cyc/row (vs 2 for
    fp32) and write an fp16 bitcast view of the fp32 psum fill tile;
    y DMAs out as fp16 and the host accumulates partials in fp32.

Device layouts (per core):
  x8T  [1024, 2048] fp8  4 x[b].T      xr8T  [1024, 2048] fp8 residual
  wq8T [1024, 512]  fp8  8 Wq[g].T     (wk8T same)
  wv8T/wvr8T [1024, 512] fp8           wo16T [512, 1024] fp16 Wo.T[g]/32
  bq   [512] f32 32 bq[g] shard        y [2048, 1024] fp16 partial out

On-chip pipeline, interleaved over 512-wide column chunks:
  - Q~/K~ [dq, T] fp16 via fp8 DoubleRow weight-stationary matmuls
    (Q kept per-window only); V [T-slice, dv] fp16 via the residual
    DoubleRow chains, stored per head with a ones column so the P@V'
    matmul also produces the softmax denominators.
  - Scores transposed per head, s_T[k, q] = K~ Q~.T (fp16, scaled 1024x).
    Score pairs land in one 2-bank PSUM tile so a single ACT instruction
    exponentiates two k-chunks (ACT per-instruction overhead matters:
    ~185ns on ~850ns of work), emitting fp16 P.
  - The causal mask is a multiplicative 0/1 square applied after exp
    (off the scores->exp critical chain).
  - P@V' in fp16 with the exp tile stationary, sub-q-outer (one PSUM
    bank per accumulation group), lagging one head behind scores/exp so
    it never waits on ACT; projection/tail work fills the PE between
    heads. 1/denominator folds into the PSUM drain (vector engine).
  - The attention output is PE-transposed (fp16) per sub-q chunk and fed
    to the fp16 W_O matmul.

Engine budget per core (cost model): PE ~157us busy (Q/K 13.7 + V 20.5,
scores 61, P@V 29, transposes 3.4, W_O 27), ACT ~157us (exp 152 + final
tail drains), DVE ~82us (psum drains, rescales), Pool ~46us (causal
masks — SBUF-only, so gpsimd can host them and they stay out of DVE's
in-order queue where they'd delay drains), DMA ~35us (7.5MB in + 4MB
out). PE and ACT are balanced co-bottlenecks at ~82% duty; modeled span
190.1us = busy + startup ramp (~9), early-window ACT starvation while
projections/V fill the PE (~9), and the endgame drain chain (~9).
Cross-window score lookahead (the PULL map) recovers most of the early
starvation in the cost model but was nondeterministically wrong on
hardware (NaNs on some runs) and is disabled.
Measured end-to-end rel err 1.245e-2 (gate 2e-2), deterministic across
runs; numpy precision model agrees to 3 digits (1.248e-2).
"""

from contextlib import ExitStack

import numpy as np

import concourse.bass as bass
import concourse.mybir as mybir
import concourse.tile as tile
from concourse import bacc
from concourse.masks import make_identity

F32 = mybir.dt.float32
F16 = mybir.dt.float16
F8 = mybir.dt.float8e4
DR = mybir.MatmulPerfMode.DoubleRow
Exp = mybir.ActivationFunctionType.Exp
Identity = mybir.ActivationFunctionType.Identity

SX = 4.0               # host scale on x8 (keeps fp8 quant relative)
SW = 8.0               # host scale on wq8/wk8 (lifts weights out of subnormals)
QK_SCALE = SX * SW     # Q~/K~ are scaled by this
EXP_SCALE = 0.125 / (QK_SCALE * QK_SCALE)

D = 1024          # model dim
T = 2048          # sequence length
BATCH = 4
NH = 16           # total heads
DH = 64           # head dim
HLOC = 8          # heads per core
DSH = 512         # value dims per core (HLOC * DH)
N_CORES = 8

TC = T // 512     # 4 column tiles of 512
KC = T // 128     # 16 k chunks of 128
DC = D // 128     # 8 contraction chunks for the QKV projections


def _build(ablate=()):
    """ablate: subset of {"pv", "exp", "scores", "mask", "rescale", "tail"}
    — drop those instruction groups (timing studies only; output garbage)."""
    nc = bacc.Bacc("TRN2", target_bir_lowering=False, debug=False,
                   num_devices=N_CORES)
    x8T = nc.dram_tensor("x8T", [D, T], F8, kind="ExternalInput").ap()
    xr8T = nc.dram_tensor("xr8T", [D, T], F8, kind="ExternalInput").ap()
    wq8T = nc.dram_tensor("wq8T", [D, DSH], F8, kind="ExternalInput").ap()
    wk8T = nc.dram_tensor("wk8T", [D, DSH], F8, kind="ExternalInput").ap()
    wv8T = nc.dram_tensor("wv8T", [D, DSH], F8, kind="ExternalInput").ap()
    wvr8T = nc.dram_tensor("wvr8T", [D, DSH], F8, kind="ExternalInput").ap()
    wo16T = nc.dram_tensor("wo16T", [DSH, D], F16, kind="ExternalInput").ap()
    bq = nc.dram_tensor("bq", [DSH], F32, kind="ExternalInput").ap()
    bk = nc.dram_tensor("bk", [DSH], F32, kind="ExternalInput").ap()
    y = nc.dram_tensor("y", [T, D], F16, kind="ExternalOutput").ap()

    with tile.TileContext(nc) as tc, ExitStack() as ctx:
        singles = ctx.enter_context(tc.tile_pool(name="singles", bufs=1))
        wpool = ctx.enter_context(tc.tile_pool(name="wpool", bufs=1))
        xtpool = ctx.enter_context(tc.tile_pool(name="xtpool", bufs=2))
        qtpool = ctx.enter_context(tc.tile_pool(name="qt", bufs=2))
        attnp = ctx.enter_context(tc.tile_pool(name="attnp", bufs=4))
        attnTp = ctx.enter_context(tc.tile_pool(name="attnTp", bufs=3))
        # exp tiles live from scores until their (queued) P@V consumes them;
        # the lookahead pend-queue peaks around 24 pair-tiles in window 2
        exp_pool = ctx.enter_context(tc.tile_pool(name="exp", bufs=32))
        small = ctx.enter_context(tc.tile_pool(name="small", bufs=8))
        ybuf = ctx.enter_context(tc.tile_pool(name="ybuf", bufs=4))
        # PSUM: 4 (two double-bank score tiles: exp reads a k-chunk PAIR in
        # one scalar-engine instruction) + 2 (PV accumulators, sub-q-outer)
        # + 2 (fill: projection groups, attn transposes, W_O groups — all
        # emission-interleaved filler work)
        ps_s = ctx.enter_context(tc.tile_pool(name="ps_s", bufs=2, space="PSUM"))
        ps_pv = ctx.enter_context(tc.tile_pool(name="ps_pv", bufs=2, space="PSUM"))
        ps_fill = ctx.enter_context(tc.tile_pool(name="ps_fill", bufs=2, space="PSUM"))

        KT_t = singles.tile([128, 4, T], F16)       # [dk%128, dk//128, t], x32
        Vp_t = singles.tile([128, KC, HLOC, DH + 1], F16)  # [t%128, t//128, h, dv+1]
        ident_t = singles.tile([128, 128], F16)
        mask_t = singles.tile([128, 128], F16)      # 0/1 causal square
        bq_t = singles.tile([128, 4], F32)
        bk_t = singles.tile([128, 4], F32)

        make_identity(nc, ident_t)
        nc.vector.memset(Vp_t[:, :, :, DH:DH + 1], 1.0)
        nc.gpsimd.memset(mask_t, 1.0)
        # s_T layout [k, q]: multiplicative 0/1 causal mask for the 128x128
        # diagonal square, applied to exp(s) AFTER the exp so the mask sits
        # off the scores->exp chain (exp(s)*0 == exp(s-1e6)). Keep 1.0
        # where (qq - kk) >= 0, else 0. (is_le is unimplemented in walrus
        # codegen, hence the negated is_ge form.)
        nc.gpsimd.affine_select(
            out=mask_t, in_=mask_t,
            compare_op=mybir.AluOpType.is_ge,
            fill=0.0,
            base=0,
            pattern=[[1, 128]],
            channel_multiplier=-1,
        )

        # Wq/Wk live c-major ([dq-chunk, contraction-chunk, col]) so the
        # first Q/K projection groups (c=0) complete after a quarter of the
        # weight bytes land — the first scores and exp start earlier
        wq_t = wpool.tile([128, 4, DC, 128], F8)
        wk_t = wpool.tile([128, 4, DC, 128], F8)
        wv_t = wpool.tile([128, DC, DSH], F8)
        wvr_t = wpool.tile([128, DC, DSH], F8)
        wo_t = wpool.tile([128, 4, D], F16)
        wq8T_r = wq8T.rearrange("(d p) (c j) -> p c d j", p=128, c=4)
        wk8T_r = wk8T.rearrange("(d p) (c j) -> p c d j", p=128, c=4)
        wv8T_r = wv8T.rearrange("(d p) j -> p d j", p=128)
        wvr8T_r = wvr8T.rearrange("(d p) j -> p d j", p=128)
        wo16T_r = wo16T.rearrange("(c p) j -> p c j", p=128)
        x8T_r = x8T.rearrange("(d p) t -> p d t", p=128)
        xr8T_r = xr8T.rearrange("(d p) t -> p d t", p=128)

        # emission order sets DMA/engine priority (each dma_start holds the
        # HWDGE descriptor generator ~625ns, so transfers are coalesced to
        # window/tensor granularity): x8(0) + first Wq/Wk block feed the
        # first scores; the V-path inputs are first needed at PV(h0)
        xt8_0 = xtpool.tile([128, DC, 512], F8, tag="xt8", name="xt8")
        nc.sync.dma_start(out=wq_t[:, 0], in_=wq8T_r[:, 0])
        nc.sync.dma_start(out=wk_t[:, 0], in_=wk8T_r[:, 0])
        # one DMA for the whole first x window: HWDGE descriptor generation
        # (625ns per dma_start, serialized) dominates the transfer time, so
        # fewer DMAs reach the first Q/K matmul sooner than split ones
        nc.sync.dma_start(out=xt8_0, in_=x8T_r[:, :, 0:512])
        nc.sync.dma_start(out=bq_t, in_=bq.rearrange("(c p) -> p c", p=128))
        nc.sync.dma_start(out=bk_t, in_=bk.rearrange("(c p) -> p c", p=128))
        for c in range(1, 4):
            nc.sync.dma_start(out=wq_t[:, c], in_=wq8T_r[:, c])
            nc.sync.dma_start(out=wk_t[:, c], in_=wk8T_r[:, c])
        xtr8_0 = xtpool.tile([128, DC, 512], F8, tag="xtr8", name="xtr8")
        nc.sync.dma_start(out=xtr8_0, in_=xr8T_r[:, :, 0:512])
        nc.sync.dma_start(out=wv_t, in_=wv8T_r)
        nc.sync.dma_start(out=wvr_t, in_=wvr8T_r)

        def proj_steps(w, box):
            """Closures emitting projection work for chunk w, finest-grain
            first: xt/qt alloc, Q groups (the attention window needs them
            first), K groups, then V groups — matching DMA data arrival so
            the PE's static instruction order never head-of-line blocks on
            a later weight load. box["qt"] is set by the first step."""
            steps = []

            def alloc(w=w):
                if w == 0:
                    xt8, xtr8 = xt8_0, xtr8_0
                else:
                    xt8 = xtpool.tile([128, DC, 512], F8, tag="xt8", name="xt8")
                    nc.sync.dma_start(out=xt8,
                                      in_=x8T_r[:, :, 512 * w:512 * (w + 1)])
                    xtr8 = xtpool.tile([128, DC, 512], F8, tag="xtr8", name="xtr8")
                    nc.sync.dma_start(out=xtr8,
                                      in_=xr8T_r[:, :, 512 * w:512 * (w + 1)])
                box["xt8"], box["xtr8"] = xt8, xtr8
                box["qt"] = qtpool.tile([128, 4, 512], F16, tag="qt", name="qt_w")
            steps.append(alloc)

            def qkstep(c, w_t, dst, bias_t, act_drain=False, w=w):
                xt8 = box["xt8"]
                psp = ps_fill.tile([128, 512], F32, tag="fill", name="psqk")
                for dp in range(DC // 2):
                    nc.tensor.matmul(
                        psp,
                        lhsT=w_t[:, c, 2 * dp:2 * dp + 2, :],
                        rhs=xt8[:, 2 * dp:2 * dp + 2, :],
                        start=(dp == 0), stop=(dp == DC // 2 - 1),
                        perf_mode=DR,
                    )
                # (an ACT Identity drain for startup parallelism was tried
                # and lost: bass lazily emits a 1.3us LoadActFuncSet before
                # the first non-Exp activation, right on the startup path)
                nc.vector.tensor_scalar_add(dst, psp, bias_t[:, c:c + 1])

            def vstep(s, w=w):
                # V in three fp8 DoubleRow chains with host-built residuals:
                # (x8+xr8)@(wv8+wvr8) minus the negligible xr8@wvr8 cross
                # term — 12 DR matmuls (3072 cyc) vs fp16's 8x512 (4096).
                # The result is 32x-scaled; Wo is host-scaled by 1/32.
                xt8, xtr8 = box["xt8"], box["xtr8"]
                psv = ps_fill.tile([128, 512], F32, tag="fill", name="psv")
                chains = ((xt8, wv_t), (xtr8, wv_t), (xt8, wvr_t))
                for dp in range(DC // 2):
                    for ci, (xa, wa) in enumerate(chains):
                        nc.tensor.matmul(
                            psv,
                            lhsT=xa[:, 2 * dp:2 * dp + 2, 128 * s:128 * (s + 1)],
                            rhs=wa[:, 2 * dp:2 * dp + 2, :],
                            start=(dp == 0 and ci == 0),
                            stop=(dp == DC // 2 - 1 and ci == 2),
                            perf_mode=DR,
                        )
                nc.vector.tensor_copy(
                    Vp_t[:, 4 * w + s, :, 0:DH],
                    psv.rearrange("p (h v) -> p h v", h=HLOC),
                )

            for c in range(4):
                steps.append(lambda c=c: qkstep(
                    c, wq_t, box["qt"][:, c, :], bq_t))
                steps.append(lambda c=c, w=w: qkstep(
                    c, wk_t, KT_t[:, c, 512 * w:512 * (w + 1)], bk_t))
            vsteps = [lambda s=s: vstep(s) for s in range(4)]
            return steps, vsteps

        def emit_scores_exp(w, h, qt_w):
            kmax = 4 * (w + 1)
            ch, po = h // 2, (h % 2) * 64
            # scores for a PAIR of k-chunks land in one 2-bank PSUM tile so
            # a single scalar-engine instruction exponentiates both (ACT
            # per-instruction overhead is the attention loop's scarcest
            # resource). All of the window's exp tiles stay live so the PV
            # loop can run sub-q-outer, one head behind.
            ex_buf = []
            for jp in range(kmax // 2):
                pssb = ps_s.tile([128, 2, 512], F32, tag="pss", name="pss")
                exb = exp_pool.tile([128, 2, 512], F16, tag="ex", name="ex")
                rel0 = 2 * jp - 4 * w
                # both matmuls write from the PAIR's first live column (the
                # second diag chunk's extra 128 columns are garbage that exp
                # covers but PV never reads — writing them keeps the paired
                # exp's input region fully initialized)
                q0 = max(rel0, 0) * 128
                for sub in range(2):
                    j = 2 * jp + sub
                    if "scores" not in ablate:
                        nc.tensor.matmul(
                            pssb[:, sub, q0:],
                            lhsT=KT_t[po:po + 64, ch, 128 * j:128 * (j + 1)],
                            rhs=qt_w[po:po + 64, ch, q0:],
                            start=True, stop=True,
                        )
                # pairs are both-full or both-diagonal (diag chunks are the
                # last 4 and 4w is even). For a diag pair the exp covers
                # [128*rel0:512] of both chunks; chunk rel0+1's columns
                # [128*rel0:128*(rel0+1)] are garbage, but PV of sub-q i
                # only reads chunks with rel <= i, so they're never used.
                e0 = max(rel0, 0) * 128
                if "exp" not in ablate:
                    nc.scalar.activation(out=exb[:, :, e0:],
                                         in_=pssb[:, :, e0:],
                                         func=Exp, scale=EXP_SCALE)
                if "mask" not in ablate:
                    for sub in range(2):
                        rel = 2 * jp + sub - 4 * w
                        if rel >= 0:
                            q0 = rel * 128
                            # zero exp(s) above the diagonal; only PV of
                            # sub-q i == rel reads this square. On gpsimd:
                            # Pool is idle and this keeps the mask out of
                            # DVE's in-order queue, where it would delay
                            # the Q/K/V psum drains behind it
                            nc.gpsimd.tensor_mul(
                                exb[:, sub, q0:q0 + 128],
                                exb[:, sub, q0:q0 + 128], mask_t)
                ex_buf.append((exb, 0))
                ex_buf.append((exb, 1))
            return ex_buf

        def emit_pv(w, h, ex_buf, attn_t, after_i=None):
            # P@V', one 128-query sub-chunk at a time: each accumulation
            # group owns one PSUM bank (bank-granular zero regions) and
            # only 2 are in flight. after_i: per-sub-q closures (the final
            # head's tail pieces) emitted right after each rescale so the
            # endgame pipeline starts before the whole head is done.
            for i in range(4):
                pso = ps_pv.tile([128, DH + 1], F32, tag="pso", name="pso")
                if "pv" not in ablate:
                    jlast = 4 * w + i
                    for j in range(jlast + 1):
                        exb, sub = ex_buf[j]
                        nc.tensor.matmul(
                            pso,
                            lhsT=exb[:, sub, 128 * i:128 * (i + 1)],
                            rhs=Vp_t[:, j, h, :],
                            start=(j == 0), stop=(j == jlast),
                        )
                if "rescale" not in ablate:
                    rec = small.tile([128, 1], F32, tag="rec", name="rec")
                    nc.vector.reciprocal(rec, pso[:, DH:DH + 1])
                    # attn = pv_psum * (1/denom), broadcast along dv
                    nc.vector.tensor_mul(
                        attn_t[:, i, DH * h:DH * (h + 1)],
                        pso[:, 0:DH],
                        rec.broadcast_to([128, DH]),
                    )
                if after_i is not None:
                    after_i[i]()

        def emit_tail_pieces(w, attn_t, use_act=False):
            """Transpose + W_O + store for window w as four per-128-query
            closures, consumed one per head so the tail never lumps between
            two heads' scores. Transposes are fp16 (1 cyc/row) into an fp16
            bitcast view of the fp32 fill tile. The final window's drains go
            to the scalar engine (idle once the last exp retires) and its y
            DMAs split per 512-chunk so the last transfer is short."""
            if "tail" in ablate:
                return []
            # final window: atT drains on ACT (idle once exp retires), ysb
            # drains stay on DVE so consecutive pipeline stages alternate
            # engines; transposes borrow the now-idle score psum pool so
            # pieces overlap instead of ping-ponging on ps_fill bufs
            drain = nc.vector.tensor_copy

            def piece(i, w=w, attn_t=attn_t):
                atT = attnTp.tile([128, 4, 128], F16, tag="attnT", name="attnT")
                if use_act:
                    pst = ps_s.tile([128, 2, 512], F32, tag="pss",
                                    name="pstE")[:, 0, :]
                else:
                    pst = ps_fill.tile([128, 512], F32, tag="fill", name="pst")
                pst16 = pst.bitcast(F16)  # [128, 1024] fp16 view
                for c in range(4):
                    nc.tensor.transpose(
                        pst16[:, 128 * c:128 * (c + 1)],
                        attn_t[:, i, 128 * c:128 * (c + 1)], ident_t)
                drain(atT, pst16[:, 0:512].rearrange("p (c q) -> p c q", c=4))
                ysb = ybuf.tile([128, 2, 512], F16, tag="ysb", name="ysb")
                for jc in range(2):
                    py = ps_fill.tile([128, 512], F32, tag="fill", name="py")
                    for c in range(4):
                        nc.tensor.matmul(
                            py,
                            lhsT=atT[:, c, :],
                            rhs=wo_t[:, c, 512 * jc:512 * (jc + 1)],
                            start=(c == 0), stop=(c == 3),
                        )
                    # final window: alternate ysb drains DVE/ACT so the two
                    # output chunks of a piece drain in parallel
                    if use_act and jc == 1:
                        nc.scalar.copy(ysb[:, jc, :], py)
                    else:
                        nc.vector.tensor_copy(ysb[:, jc, :], py)
                    if use_act:
                        nc.sync.dma_start(
                            out=y[512 * w + 128 * i:512 * w + 128 * (i + 1),
                                  512 * jc:512 * (jc + 1)],
                            in_=ysb[:, jc, :],
                        )
                if not use_act:
                    nc.sync.dma_start(
                        out=y[512 * w + 128 * i:512 * w + 128 * (i + 1), :],
                        in_=ysb.rearrange("p j q -> p (j q)"),
                    )
            return [lambda i=i: piece(i) for i in range(4)]

        # Driver: a softly-pipelined schedule over 32 (window, head) tasks.
        # P@V trails scores/exp through a pend QUEUE; windows 0/1 pull the
        # next window's first heads' scores+exp into their last slots so the
        # scalar engine's in-order exp stream never starves across window
        # boundaries (early windows are PE-bound, late ones ACT-bound). The
        # queue debt is repaid with double flushes in the late-w2/w3 slots,
        # where exp is long and the PE idles. PE filler placement: Q/K
        # projections for w+1 spread evenly over w's slots; V(w) groups run
        # at w's first two slots (before PV(w,h0)'s flush — PE executes in
        # emission order, so V(w) must precede it); tail pieces defer into
        # windows >= 2.
        PULL = {}              # lookahead disabled: pulls raced on hardware
        box0 = {}
        qk0, v0 = proj_steps(0, box0)
        for s in qk0:          # alloc, Q x4, K x4
            s()
        qt_map = {0: box0["qt"]}
        attn_map = {}
        boxes = {}
        pendq = []             # (w, h, ex_buf, attn_t) awaiting PV, FIFO
        v_now = v0             # V groups for the current window
        v_next = []
        qk_carry = []
        tailq = []             # pending per-sub-q tail closures

        def get_attn(wi):
            if wi not in attn_map:
                attn_map[wi] = attnp.tile([128, 4, DSH], F16, tag="attn",
                                          name="attn_t")
            return attn_map[wi]

        def flush_one():
            pw, ph, pex, pat = pendq.pop(0)
            emit_pv(pw, ph, pex, pat)
            if ph == HLOC - 1:           # window pw fully rescaled
                tailq.extend(emit_tail_pieces(pw, pat))

        for w in range(TC):
            start_h = PULL.get(w - 1, 0)
            own = list(range(start_h, HLOC))
            pulls = list(range(PULL.get(w, 0))) if w + 1 < TC else []
            nslots = len(own)
            qk_steps = list(qk_carry)
            qk_carry = []
            if w == 0:
                qk_steps.append(lambda: nc.sync.dma_start(
                    out=wo_t, in_=wo16T_r))
            if w + 1 < TC:
                box = {}
                nqk, v_next = proj_steps(w + 1, box)
                qk_steps += nqk
                boxes[w + 1] = box
            it = iter(qk_steps)
            n_qk = len(qk_steps)
            done = 0
            for idx, h in enumerate(own):
                last_task = (w == TC - 1 and h == HLOC - 1)
                ex = emit_scores_exp(w, h, qt_map[w])
                # V(w) spreads 2,1,1 over slots 0-2: skipping the slot-1 PV
                # flush (below) pushes PV(w,0) to slot 2, so slot 1 sheds a
                # V group from the PE backlog that delays the exp stream in
                # the PE-bound early windows
                for s in v_now[:1]:      # V(w): one group per slot, 0-3
                    s()
                v_now = v_now[1:]
                target = n_qk * (idx + 1) / nslots
                while done < target:
                    s = next(it, None)
                    if s is None:
                        break
                    s()
                    done += 1
                if last_task:
                    nf = len(pendq)      # drain everything before the tail
                elif idx in (1, 2):
                    # defer PV(w,0) to slot 3: V(w)#4 (emitted at slot 3,
                    # before the flush) must precede it in PE program order
                    # — real silicon only reorders Ldweights, not matmuls
                    nf = 0
                elif idx >= HLOC - 2:
                    nf = 2               # repay the deferred flushes
                else:
                    nf = 1
                for _ in range(nf):
                    if pendq:
                        flush_one()
                if w >= 2 and tailq:
                    tailq.pop(0)()
                if last_task:
                    # final head: pipeline its PV with the window's tail
                    emit_pv(w, h, ex, get_attn(w),
                            after_i=emit_tail_pieces(w, get_attn(w),
                                                     use_act=True) or None)
                else:
                    pendq.append((w, h, ex, get_attn(w)))
                pi = idx - (nslots - len(pulls))
                if 0 <= pi < len(pulls):
                    qt_map[w + 1] = boxes[w + 1]["qt"]
                    ex2 = emit_scores_exp(w + 1, pulls[pi], qt_map[w + 1])
                    pendq.append((w + 1, pulls[pi], ex2, get_attn(w + 1)))
            v_now = v_next
            v_next = []
            if w + 1 < TC:
                qt_map[w + 1] = boxes[w + 1]["qt"]
        for _ in range(len(pendq)):
            flush_one()
        for t in tailq:
            t()
    nc.compile()
    return nc


def shard_inputs(x, Wq, bq, Wk, bk, Wv, bv, Wo, bo):
    """Returns the 8 per-core input maps (host-side dtype/layout prep)."""
    import ml_dtypes

    F8NP = ml_dtypes.float8_e4m3
    in_maps = []
    for c in range(N_CORES):
        b, g = c // 2, c % 2
        sl = slice(DSH * g, DSH * (g + 1))
        xT = np.ascontiguousarray(x[b].T) * SX
        x8 = xT.astype(F8NP)
        wvs = np.ascontiguousarray(Wv[sl, :].T) * SW
        wv8 = wvs.astype(F8NP)
        in_maps.append({
            "x8T": x8,
            "xr8T": (xT - x8.astype(np.float32)).astype(F8NP),
            "wq8T": np.ascontiguousarray(Wq[sl, :].T * SW).astype(F8NP),
            "wk8T": np.ascontiguousarray(Wk[sl, :].T * SW).astype(F8NP),
            "wv8T": wv8,
            "wvr8T": (wvs - wv8.astype(np.float32)).astype(F8NP),
            # Wo pre-divided by the V-path scale (SX*SW) so attn rides
            # 32x-scaled through PV and the tail
            "wo16T": np.ascontiguousarray(Wo.T[sl, :] / (SX * SW)).astype(
                np.float16),
            "bq": np.ascontiguousarray(bq[sl] * QK_SCALE).astype(np.float32),
            "bk": np.ascontiguousarray(bk[sl] * QK_SCALE).astype(np.float32),
        })
    return in_maps


def combine_outputs(results, bv, Wo, bo):
    """Sum head-group partials per batch + rank-1 bias corrections."""
    corr = (bv @ Wo.T + bo).astype(np.float32)  # [D]; exact because softmax
    y = np.empty((BATCH, T, D), dtype=np.float32)  # rows sum to 1
    for b in range(BATCH):
        y[b] = (results[2 * b]["y"].astype(np.float32)
                + results[2 * b + 1]["y"].astype(np.float32) + corr)
    return y


def run_sharded(inputs, trace=False):
    """Build, compile, run on cores 0-7. Returns (y_full, BassKernelResults)."""
    from concourse import bass_utils

    inputs = {k: np.asarray(v, dtype=np.float32) for k, v in inputs.items()}
    nc = _build()
    in_maps = shard_inputs(
        inputs["x"], inputs["Wq"], inputs["bq"], inputs["Wk"], inputs["bk"],
        inputs["Wv"], inputs["bv"], inputs["Wo"], inputs["bo"])
    res = bass_utils.run_bass_kernel_spmd(
        nc, in_maps, list(range(N_CORES)), trace=trace)
    y = combine_outputs(res.results, inputs["bv"], inputs["Wo"], inputs["bo"])
    return y, res


def kernel(**inputs):
    y, _ = run_sharded(inputs, trace=False)
    return y


if __name__ == "__main__":
    rng = np.random.default_rng(0)
    demo = {
        "x": rng.standard_normal((BATCH, T, D), dtype=np.float32),
        "Wq": rng.standard_normal((D, D), dtype=np.float32) * 0.02,
        "bq": np.zeros(D, np.float32),
        "Wk": rng.standard_normal((D, D), dtype=np.float32) * 0.02,
        "bk": np.zeros(D, np.float32),
        "Wv": rng.standard_normal((D, D), dtype=np.float32) * 0.02,
        "bv": np.zeros(D, np.float32),
        "Wo": rng.standard_normal((D, D), dtype=np.float32) * 0.02,
        "bo": np.zeros(D, np.float32),
    }
    out = kernel(**demo)
    print(out.shape, out.dtype)
